# revision 52
# baseline (speedup 1.0000x reference)
"""Causal self-attention (B=2, S=2048, D=2048, H=16) on 8 trn2 NeuronCores.

Sharding: core c -> batch b = c//4, head-group hg = c%4 (4 heads of 128 dims).
Each core computes its heads' attention plus the partial output projection
(row-parallel split of W_proj); the host sums the 4 partials per batch.

v2 design (vs the f32r/DRAM-scratch baseline):
- All matmul operands in bf16 (same 1 cycle/row PE rate as f32r, half the
  DMA bytes and SBUF footprint, and no >=256 moving-col constraint so the
  causal diagonal tiles compute only live columns).
- Everything SBUF-resident: x, weights, Q^T/K^T/V, attention outputs. No
  DRAM scratch roundtrip at all; the only DMA is ~15MB in + 16.8MB y out.
- The softmax denominator is NOT a per-k-tile PE matmul (that costs a full
  second stream of e through the PE, ~30us/core). Instead the Vector engine
  accumulates e tiles (e_acc += e) as they are produced and one 512-col
  ones-matmul per (head, q-group) reduces e_acc.
- Interleaved emission: a filler queue of projection / V / output-projection
  matmul groups is drained between attention iterations so the PE never
  idles waiting on the Scalar engine's exp.
"""

import sys

sys.path.insert(0, "/opt/trn_rl_repo")

from collections import deque
from contextlib import ExitStack

import numpy as np
import ml_dtypes

import concourse.bass as bass
import concourse.mybir as mybir
import concourse.tile as tile
from concourse import bacc
from concourse.bass_utils import run_bass_kernel_spmd

B, S, D, H = 2, 2048, 2048, 16
HD = D // H  # 128
NH = 4  # heads per core
HG = H // NH  # head groups = 4
P = 128
KT = D // P  # 16 k-tiles over model dim
NQ = 4  # q-groups of 512
QW = S // NQ  # 512
ST = S // P  # 16 token-tiles of 128
SCALE = float(1.0 / np.sqrt(D).astype(np.float32))
MASK_NEG = -30000.0  # exp(SCALE * -30000) == 0 in fp32

F32 = mybir.dt.float32
BF16 = mybir.dt.bfloat16
FP8 = mybir.dt.float8e4
BF = ml_dtypes.bfloat16
F8 = ml_dtypes.float8_e4m3
WS = 64.0  # fp8 weight prescale; folded back out via the exp() scale




def build_bass():
    nc = bacc.Bacc("TRN2")

    xT = nc.declare_dram_parameter("xT", [D, S], BF16, isOutput=False)
    x8T = nc.declare_dram_parameter("x8T", [D, S], FP8, isOutput=False)
    wq8 = nc.declare_dram_parameter("wq8", [D, NH * HD], FP8, isOutput=False)
    wk8 = nc.declare_dram_parameter("wk8", [D, NH * HD], FP8, isOutput=False)
    wv = nc.declare_dram_parameter("wv", [D, NH * HD], BF16, isOutput=False)
    wp = nc.declare_dram_parameter("wp", [NH * HD, D], BF16, isOutput=False)
    mask = nc.declare_dram_parameter("mask", [P, P], F32, isOutput=False)
    y = nc.declare_dram_parameter("y", [S, D], BF16, isOutput=True)

    with tile.TileContext(nc) as tc, ExitStack() as top:
        const = top.enter_context(tc.tile_pool(name="const", bufs=1))
        xpool = top.enter_context(tc.tile_pool(name="xpool", bufs=1))
        wpool = top.enter_context(tc.tile_pool(name="wpool", bufs=1))
        qkpool = top.enter_context(tc.tile_pool(name="qkpool", bufs=1))
        vpool = top.enter_context(tc.tile_pool(name="vpool", bufs=1))
        apool = top.enter_context(tc.tile_pool(name="apool", bufs=1))
        epool = top.enter_context(tc.tile_pool(name="epool", bufs=4))
        eaccp = top.enter_context(tc.tile_pool(name="eaccp", bufs=2))
        rpool = top.enter_context(tc.tile_pool(name="rpool", bufs=1))
        rbpool = top.enter_context(tc.tile_pool(name="rbpool", bufs=1))
        ybp = top.enter_context(tc.tile_pool(name="ybp", bufs=4))
        fpsum = top.enter_context(tc.tile_pool(name="fpsum", bufs=2, space="PSUM"))
        spsum = top.enter_context(tc.tile_pool(name="spsum", bufs=4, space="PSUM"))
        upsum = top.enter_context(tc.tile_pool(name="upsum", bufs=1, space="PSUM"))
        dpsum = top.enter_context(tc.tile_pool(name="dpsum", bufs=1, space="PSUM"))

        mask_sb = const.tile([P, P], F32)
        ones_bf = const.tile([P, 1], BF16)
        nc.vector.memset(ones_bf, 1.0)
        warm_sb = const.tile([P, QW], BF16)
        nc.vector.memset(warm_sb, 0.0)

        x8_sb = xpool.tile([P, KT, S], FP8, tag="x8")
        wq_sb = wpool.tile([P, KT, NH * HD], FP8, tag="wq")
        wk_sb = wpool.tile([P, KT, NH * HD], FP8, tag="wk")
        wv_sb = wpool.tile([P, KT, NH * HD], BF16, tag="wv")
        wp_sb = wpool.tile([P, NH, D], BF16, tag="wp")
        qt_sb = qkpool.tile([P, NH, S], BF16, tag="qt")
        kt_sb = qkpool.tile([P, NH, S], BF16, tag="kt")
        v_sb = vpool.tile([P, ST, NH * HD], BF16)
        a_sb = apool.tile([P, NH, S], BF16)
        # bf16 x streams through a rotating window, one tile per V chain
        xwpool = top.enter_context(tc.tile_pool(name="xwpool", bufs=6))
        xw_tiles = [
            xwpool.tile([P, KT, P], BF16, tag="xw", name=f"xw_{m}")
            for m in range(ST)
        ]

        xT_r = xT[:, :].rearrange("(k p) s -> p k s", p=P)
        x8_r = x8T[:, :].rearrange("(k p) s -> p k s", p=P)
        wq_r = wq8[:, :].rearrange("(k p) m -> p k m", p=P)
        wk_r = wk8[:, :].rearrange("(k p) m -> p k m", p=P)
        wv_r = wv[:, :].rearrange("(k p) m -> p k m", p=P)
        wp_r = wp[:, :].rearrange("(h p) n -> p h n", p=P)

        # DMA issue order = priority order on the SP queue (engines execute
        # in order, so issue order must match need order to avoid
        # head-of-line blocking on the rotating xw windows).
        nc.sync.dma_start(x8_sb[:, 0:8, 0:QW], x8_r[:, 0:8, 0:QW])
        nc.sync.dma_start(wq_sb[:, 0:8, :], wq_r[:, 0:8, :])
        nc.sync.dma_start(mask_sb, mask[:, :])
        nc.sync.dma_start(x8_sb[:, 8:KT, 0:QW], x8_r[:, 8:KT, 0:QW])
        nc.sync.dma_start(wq_sb[:, 8:KT, :], wq_r[:, 8:KT, :])
        nc.sync.dma_start(wk_sb[:, 0:8, :], wk_r[:, 0:8, :])
        nc.sync.dma_start(wk_sb[:, 8:KT, :], wk_r[:, 8:KT, :])
        nc.sync.dma_start(wv_sb, wv_r)
        for tb in range(1, NQ):
            ts = slice(tb * QW, (tb + 1) * QW)
            for m in range(4 * (tb - 1), 4 * tb):
                nc.sync.dma_start(xw_tiles[m], xT_r[:, :, m * P : (m + 1) * P])
            nc.sync.dma_start(x8_sb[:, :, ts], x8_r[:, :, ts])
        for m in range(4 * (NQ - 1), ST):
            nc.sync.dma_start(xw_tiles[m], xT_r[:, :, m * P : (m + 1) * P])
        nc.sync.dma_start(wp_sb, wp_r)

        # PE p-state warmup: the first real matmul waits ~14us for input DMA;
        # keep the otherwise-idle PE busy on throwaway matmuls so the clock
        # is fully ramped (and stays ramped) when real work arrives.
        warm_ps = dpsum.tile([1, QW], F32, tag="d")
        for _ in range(56):
            nc.tensor.matmul(warm_ps, lhsT=ones_bf, rhs=warm_sb, start=True, stop=True)

        # ---- filler queue: PE work drained between attention iterations ----
        filler = deque()
        done = set()

        def push(key, fn):
            filler.append((key, fn))

        def drain(n=1):
            for _ in range(min(n, len(filler))):
                key, fn = filler.popleft()
                fn()
                done.add(key)

        def force(key):
            while key not in done and filler:
                drain(1)

        # Projection chains are split into sub-items so the filler queue can
        # be drained at a fine grain between attention iterations; the PSUM
        # tile is shared across a chain's sub-items via a cell.
        # q/k projections run as fp8 DoubleRow matmuls: two 128-row k-tiles
        # per instruction at 0.5 cycles/row (2x PE rate).
        def qk_subs(h, w_sb, dst, tg):
            cell = [None]
            def sub(kc):
                def go():
                    if kc == 0:
                        cell[0] = fpsum.tile(
                            [P, QW], F32, tag="f", name=f"qk_{h}_{tg}"
                        )
                    ps = cell[0]
                    for kp in range(kc, kc + 8, 2):
                        nc.tensor.matmul(
                            ps,
                            lhsT=w_sb[:, kp : kp + 2, h * HD : (h + 1) * HD],
                            rhs=x8_sb[:, kp : kp + 2, tg * QW : (tg + 1) * QW],
                            start=(kp == 0),
                            stop=(kp == KT - 2),
                            perf_mode=mybir.MatmulPerfMode.DoubleRow,
                        )
                    if kc == KT - 8:
                        nc.scalar.copy(dst[:, h, tg * QW : (tg + 1) * QW], ps)
                return go
            return [sub(kc) for kc in range(0, KT, 8)]

        def v_subs(m):
            cell = [None]
            def sub(kc):
                def go():
                    if kc == 0:
                        cell[0] = fpsum.tile([P, QW], F32, tag="f", name=f"v_{m}")
                    ps = cell[0]
                    for k in range(kc, kc + 4):
                        nc.tensor.matmul(
                            ps,
                            lhsT=xw_tiles[m][:, k, :],
                            rhs=wv_sb[:, k, :],
                            start=(k == 0),
                            stop=(k == KT - 1),
                        )
                    if kc == KT - 4:
                        nc.scalar.copy(v_sb[:, m, :], ps)
                return go
            return [sub(kc) for kc in range(0, KT, 4)]

        def p3_tile(m, n):
            def go():
                ps = fpsum.tile([P, QW], F32, tag="f")
                for h in range(NH):
                    nc.tensor.matmul(
                        ps,
                        lhsT=a_sb[:, h, m * P : (m + 1) * P],
                        rhs=wp_sb[:, h, n * QW : (n + 1) * QW],
                        start=(h == 0),
                        stop=(h == NH - 1),
                    )
                yb = ybp.tile([P, QW], BF16, tag="yb")
                # alternate the bounce copy between Scalar and Vector so the
                # final drain isn't serialized on one engine
                if (m + n) % 2 == 0:
                    nc.scalar.copy(yb, ps)
                else:
                    nc.vector.tensor_copy(yb, ps)
                nc.sync.dma_start(y[m * P : (m + 1) * P, n * QW : (n + 1) * QW], yb)
            return go

        def push_subs(key, subs):
            for i, s in enumerate(subs):
                push(key if i == len(subs) - 1 else key + ("sub", i), s)

        def push_qk(h):
            for tg in range(NQ):
                push_subs(("qk", h, tg, 0), qk_subs(h, wq_sb, qt_sb, tg))
                push_subs(("qk", h, tg, 1), qk_subs(h, wk_sb, kt_sb, tg))

        # The previous block's denominator matmul + normalize is deferred
        # until after the next block's first scores, so the PE doesn't stall
        # on the tail of the exp/accumulate pipeline at block boundaries.
        pending_fin = [None]

        def run_fin():
            if pending_fin[0] is not None:
                fn = pending_fin[0]
                pending_fin[0] = None
                fn()

        def attn_block(h, qg):
            kmax = 4 * qg + 4
            qs0 = qg * QW
            e_acc = eaccp.tile([P, QW], BF16, tag="eacc")
            ups = upsum.tile([P, QW], F32, tag="u")
            sps_tiles = [None] * kmax
            e_tiles = [None] * kmax

            def emit_score(kt):
                r = kt - 4 * qg
                c0 = 0 if r < 0 else r * P
                sps = spsum.tile([P, QW], F32, tag="s")
                nc.tensor.matmul(
                    sps[:, c0:],
                    lhsT=kt_sb[:, h, kt * P : (kt + 1) * P],
                    rhs=qt_sb[:, h, qs0 + c0 : qs0 + QW],
                    start=True,
                    stop=True,
                )
                sps_tiles[kt] = (sps, c0, r)

            def emit_post(kt):
                sps, c0, r = sps_tiles[kt]
                if r >= 0:
                    nc.vector.tensor_tensor(
                        sps[:, c0 : c0 + P], sps[:, c0 : c0 + P], mask_sb,
                        op=mybir.AluOpType.add,
                    )
                e = epool.tile([P, QW], BF16, tag="e")
                nc.scalar.activation(
                    e[:, c0:], sps[:, c0:],
                    mybir.ActivationFunctionType.Exp, scale=SCALE / (WS * WS),
                )
                if kt == 0:
                    nc.vector.tensor_copy(e_acc, e)
                else:
                    nc.vector.tensor_tensor(
                        e_acc[:, c0:], e_acc[:, c0:], e[:, c0:],
                        op=mybir.AluOpType.add,
                    )
                e_tiles[kt] = (e, c0)

            def emit_av(kt):
                if h == 0:
                    # first head of the wave: pull in the v chain this AV
                    # reads, one tok-tile at a time, so wave-0 attention can
                    # start as soon as v(0) lands instead of after v(0..3)
                    force(("v", kt))
                e, c0 = e_tiles[kt]
                nc.tensor.matmul(
                    ups[:, c0:],
                    lhsT=v_sb[:, kt, h * HD : (h + 1) * HD],
                    rhs=e[:, c0:],
                    start=(kt == 0),
                    stop=(kt == kmax - 1),
                )

            for kt in range(min(3, kmax)):
                emit_score(kt)
                emit_post(kt)
            run_fin()
            for kt in range(kmax):
                if kt + 3 < kmax:
                    emit_score(kt + 3)
                    emit_post(kt + 3)
                emit_av(kt)
                if kt % 2 == 1:
                    drain(1)

            def fin():
                dps = dpsum.tile([1, QW], F32, tag="d")
                nc.tensor.matmul(dps, lhsT=ones_bf, rhs=e_acc, start=True, stop=True)
                rcp = rpool.tile([1, QW], F32)
                nc.vector.reciprocal_approx_fast(rcp, dps)
                rb = rbpool.tile([P, QW], F32)
                nc.gpsimd.partition_broadcast(rb, rcp)
                nc.vector.tensor_tensor(
                    a_sb[:, h, qs0 : qs0 + QW], ups, rb, op=mybir.AluOpType.mult
                )

            pending_fin[0] = fin

        # ---- emission ----
        # Wave order: for each q-group, run all 4 heads' attention blocks.
        # This spreads the V-projection and output-projection filler across
        # the whole run (v tok-tiles 4qg..4qg+3 feed wave qg; p3 for wave qg
        # becomes available during wave qg+1). Attention blocks force what
        # they need (qt/kt tokgroups <= qg, v tok-tiles <= 4qg+3) and
        # pace-drain the rest.
        for tg in range(NQ):
            for h in range(NH):
                push_subs(("qk", h, tg, 0), qk_subs(h, wq_sb, qt_sb, tg))
                push_subs(("qk", h, tg, 1), qk_subs(h, wk_sb, kt_sb, tg))
            for m in range(4 * tg, 4 * tg + 4):
                push_subs(("v", m), v_subs(m))

        def push_p3(qg):
            for m in range(4 * qg, 4 * qg + 4):
                for n in range(NQ):
                    push(("p3", m, n), p3_tile(m, n))

        for qg in range(NQ):
            for h in range(NH):
                force(("qk", h, qg, 1))
                attn_block(h, qg)
                if h == 0 and qg > 0:
                    # fin of wave qg-1's last block just ran inside this
                    # block; wave qg-1's a_sb rows are now final
                    push_p3(qg - 1)
        run_fin()
        push_p3(NQ - 1)
        drain(len(filler))

    nc.finalize()
    return nc


def _build_mask():
    # triangular block mask for the diagonal score tiles: scores[k_row,
    # q_col] allowed iff q >= k. Applied pre-scale — scores carry the WS^2
    # fp8 prescale, so the mask does too: exp(SCALE/WS^2 * (s + mask)).
    k = np.arange(P)[:, None]
    c = np.arange(P)[None, :]
    return np.where(c >= k, 0.0, MASK_NEG * WS * WS).astype(np.float32)


_NC_CACHE = {}


def _get_nc():
    if "nc" not in _NC_CACHE:
        _NC_CACHE["nc"] = build_bass()
    return _NC_CACHE["nc"]


def make_in_maps(x, W_qkv, W_proj):
    x = np.asarray(x, dtype=np.float32)
    W_qkv = np.asarray(W_qkv, dtype=np.float32)
    W_proj = np.asarray(W_proj, dtype=np.float32)
    Wq, Wk, Wv = W_qkv[0:D], W_qkv[D : 2 * D], W_qkv[2 * D : 3 * D]
    mask = _build_mask()
    xT_b = [np.ascontiguousarray(x[b].T) for b in range(B)]
    xbf_b = [t.astype(BF) for t in xT_b]
    x8_b = [t.astype(F8) for t in xT_b]
    in_maps = []
    for c in range(8):
        b, hg = c // HG, c % HG
        rows = slice(hg * NH * HD, (hg + 1) * NH * HD)
        in_maps.append(
            {
                "xT": xbf_b[b],
                "x8T": x8_b[b],
                "wq8": np.ascontiguousarray(WS * Wq[rows].T).astype(F8),
                "wk8": np.ascontiguousarray(WS * Wk[rows].T).astype(F8),
                "wv": np.ascontiguousarray(Wv[rows].T).astype(BF),
                "wp": np.ascontiguousarray(W_proj[:, rows].T).astype(BF),
                "mask": mask,
            }
        )
    return in_maps


def run(x, W_qkv, W_proj, trace=False):
    nc = _get_nc()
    in_maps = make_in_maps(x, W_qkv, W_proj)
    res = run_bass_kernel_spmd(nc, in_maps, core_ids=list(range(8)), trace=trace)
    out = np.zeros((B, S, D), dtype=np.float32)
    for c in range(8):
        out[c // HG] += res.results[c]["y"].astype(np.float32)
    return out, res


def kernel(x, W_qkv, W_proj):
    out, _ = run(x, W_qkv, W_proj, trace=False)
    return out


# revision 54
# speedup vs baseline: 1.0027x; 1.0027x over previous
"""Causal self-attention (B=2, S=2048, D=2048, H=16) on 8 trn2 NeuronCores.

Sharding: core c -> batch b = c//4, head-group hg = c%4 (4 heads of 128 dims).
Each core computes its heads' attention plus the partial output projection
(row-parallel split of W_proj); the host sums the 4 partials per batch.

v2 design (vs the f32r/DRAM-scratch baseline):
- All matmul operands in bf16 (same 1 cycle/row PE rate as f32r, half the
  DMA bytes and SBUF footprint, and no >=256 moving-col constraint so the
  causal diagonal tiles compute only live columns).
- Everything SBUF-resident: x, weights, Q^T/K^T/V, attention outputs. No
  DRAM scratch roundtrip at all; the only DMA is ~15MB in + 16.8MB y out.
- The softmax denominator is NOT a per-k-tile PE matmul (that costs a full
  second stream of e through the PE, ~30us/core). Instead the Vector engine
  accumulates e tiles (e_acc += e) as they are produced and one 512-col
  ones-matmul per (head, q-group) reduces e_acc.
- Interleaved emission: a filler queue of projection / V / output-projection
  matmul groups is drained between attention iterations so the PE never
  idles waiting on the Scalar engine's exp.
"""

import sys

sys.path.insert(0, "/opt/trn_rl_repo")

from collections import deque
from contextlib import ExitStack

import numpy as np
import ml_dtypes

import concourse.bass as bass
import concourse.mybir as mybir
import concourse.tile as tile
from concourse import bacc
from concourse.bass_utils import run_bass_kernel_spmd

B, S, D, H = 2, 2048, 2048, 16
HD = D // H  # 128
NH = 4  # heads per core
HG = H // NH  # head groups = 4
P = 128
KT = D // P  # 16 k-tiles over model dim
NQ = 4  # q-groups of 512
QW = S // NQ  # 512
ST = S // P  # 16 token-tiles of 128
SCALE = float(1.0 / np.sqrt(D).astype(np.float32))
MASK_NEG = -30000.0  # exp(SCALE * -30000) == 0 in fp32

F32 = mybir.dt.float32
BF16 = mybir.dt.bfloat16
FP8 = mybir.dt.float8e4
BF = ml_dtypes.bfloat16
F8 = ml_dtypes.float8_e4m3
WS = 64.0  # fp8 weight prescale; folded back out via the exp() scale




def build_bass():
    nc = bacc.Bacc("TRN2")

    xT = nc.declare_dram_parameter("xT", [D, S], BF16, isOutput=False)
    x8T = nc.declare_dram_parameter("x8T", [D, S], FP8, isOutput=False)
    wq8 = nc.declare_dram_parameter("wq8", [D, NH * HD], FP8, isOutput=False)
    wk8 = nc.declare_dram_parameter("wk8", [D, NH * HD], FP8, isOutput=False)
    wv = nc.declare_dram_parameter("wv", [D, NH * HD], BF16, isOutput=False)
    wp = nc.declare_dram_parameter("wp", [NH * HD, D], BF16, isOutput=False)
    mask = nc.declare_dram_parameter("mask", [P, P], F32, isOutput=False)
    y = nc.declare_dram_parameter("y", [S, D], BF16, isOutput=True)

    with tile.TileContext(nc) as tc, ExitStack() as top:
        const = top.enter_context(tc.tile_pool(name="const", bufs=1))
        xpool = top.enter_context(tc.tile_pool(name="xpool", bufs=1))
        wpool = top.enter_context(tc.tile_pool(name="wpool", bufs=1))
        qkpool = top.enter_context(tc.tile_pool(name="qkpool", bufs=1))
        vpool = top.enter_context(tc.tile_pool(name="vpool", bufs=1))
        apool = top.enter_context(tc.tile_pool(name="apool", bufs=1))
        epool = top.enter_context(tc.tile_pool(name="epool", bufs=4))
        eaccp = top.enter_context(tc.tile_pool(name="eaccp", bufs=2))
        rpool = top.enter_context(tc.tile_pool(name="rpool", bufs=1))
        rbpool = top.enter_context(tc.tile_pool(name="rbpool", bufs=1))
        ybp = top.enter_context(tc.tile_pool(name="ybp", bufs=4))
        fpsum = top.enter_context(tc.tile_pool(name="fpsum", bufs=2, space="PSUM"))
        spsum = top.enter_context(tc.tile_pool(name="spsum", bufs=4, space="PSUM"))
        upsum = top.enter_context(tc.tile_pool(name="upsum", bufs=1, space="PSUM"))
        dpsum = top.enter_context(tc.tile_pool(name="dpsum", bufs=1, space="PSUM"))

        mask_sb = const.tile([P, P], F32)
        ones_bf = const.tile([P, 1], BF16)
        nc.vector.memset(ones_bf, 1.0)
        warm_sb = const.tile([P, QW], BF16)
        nc.vector.memset(warm_sb, 0.0)

        x8_sb = xpool.tile([P, KT, S], FP8, tag="x8")
        wq_sb = wpool.tile([P, KT, NH * HD], FP8, tag="wq")
        wk_sb = wpool.tile([P, KT, NH * HD], FP8, tag="wk")
        wv_sb = wpool.tile([P, KT, NH * HD], BF16, tag="wv")
        wp_sb = wpool.tile([P, NH, D], BF16, tag="wp")
        qt_sb = qkpool.tile([P, NH, S], BF16, tag="qt")
        kt_sb = qkpool.tile([P, NH, S], BF16, tag="kt")
        v_sb = vpool.tile([P, ST, NH * HD], BF16)
        a_sb = apool.tile([P, NH, S], BF16)
        # bf16 x streams through a rotating window, one tile per V chain
        xwpool = top.enter_context(tc.tile_pool(name="xwpool", bufs=6))
        xw_tiles = [
            xwpool.tile([P, KT, P], BF16, tag="xw", name=f"xw_{m}")
            for m in range(ST)
        ]

        xT_r = xT[:, :].rearrange("(k p) s -> p k s", p=P)
        x8_r = x8T[:, :].rearrange("(k p) s -> p k s", p=P)
        wq_r = wq8[:, :].rearrange("(k p) m -> p k m", p=P)
        wk_r = wk8[:, :].rearrange("(k p) m -> p k m", p=P)
        wv_r = wv[:, :].rearrange("(k p) m -> p k m", p=P)
        wp_r = wp[:, :].rearrange("(h p) n -> p h n", p=P)

        # DMA issue order = priority order on the SP queue (engines execute
        # in order, so issue order must match need order to avoid
        # head-of-line blocking on the rotating xw windows).
        nc.sync.dma_start(x8_sb[:, 0:8, 0:QW], x8_r[:, 0:8, 0:QW])
        nc.sync.dma_start(wq_sb[:, 0:8, :], wq_r[:, 0:8, :])
        nc.sync.dma_start(mask_sb, mask[:, :])
        nc.sync.dma_start(x8_sb[:, 8:KT, 0:QW], x8_r[:, 8:KT, 0:QW])
        nc.sync.dma_start(wq_sb[:, 8:KT, :], wq_r[:, 8:KT, :])
        nc.sync.dma_start(wk_sb[:, 0:8, :], wk_r[:, 0:8, :])
        nc.sync.dma_start(wk_sb[:, 8:KT, :], wk_r[:, 8:KT, :])
        nc.sync.dma_start(wv_sb, wv_r)
        for tb in range(1, NQ):
            ts = slice(tb * QW, (tb + 1) * QW)
            for m in range(4 * (tb - 1), 4 * tb):
                nc.sync.dma_start(xw_tiles[m], xT_r[:, :, m * P : (m + 1) * P])
            nc.sync.dma_start(x8_sb[:, :, ts], x8_r[:, :, ts])
        for m in range(4 * (NQ - 1), ST):
            nc.sync.dma_start(xw_tiles[m], xT_r[:, :, m * P : (m + 1) * P])
        nc.sync.dma_start(wp_sb, wp_r)

        # PE p-state warmup: the first real matmul waits ~14us for input DMA;
        # keep the otherwise-idle PE busy on throwaway matmuls so the clock
        # is fully ramped (and stays ramped) when real work arrives.
        warm_ps = dpsum.tile([1, QW], F32, tag="d")
        for _ in range(56):
            nc.tensor.matmul(warm_ps, lhsT=ones_bf, rhs=warm_sb, start=True, stop=True)

        # ---- filler queue: PE work drained between attention iterations ----
        filler = deque()
        done = set()

        def push(key, fn):
            filler.append((key, fn))

        def drain(n=1):
            for _ in range(min(n, len(filler))):
                key, fn = filler.popleft()
                fn()
                done.add(key)

        def force(key):
            while key not in done and filler:
                drain(1)

        # Projection chains are split into sub-items so the filler queue can
        # be drained at a fine grain between attention iterations; the PSUM
        # tile is shared across a chain's sub-items via a cell.
        # q/k projections run as fp8 DoubleRow matmuls: two 128-row k-tiles
        # per instruction at 0.5 cycles/row (2x PE rate).
        def qk_subs(h, w_sb, dst, tg):
            cell = [None]
            def sub(kc):
                def go():
                    if kc == 0:
                        cell[0] = fpsum.tile(
                            [P, QW], F32, tag="f", name=f"qk_{h}_{tg}"
                        )
                    ps = cell[0]
                    for kp in range(kc, kc + 8, 2):
                        nc.tensor.matmul(
                            ps,
                            lhsT=w_sb[:, kp : kp + 2, h * HD : (h + 1) * HD],
                            rhs=x8_sb[:, kp : kp + 2, tg * QW : (tg + 1) * QW],
                            start=(kp == 0),
                            stop=(kp == KT - 2),
                            perf_mode=mybir.MatmulPerfMode.DoubleRow,
                        )
                    if kc == KT - 8:
                        nc.scalar.copy(dst[:, h, tg * QW : (tg + 1) * QW], ps)
                return go
            return [sub(kc) for kc in range(0, KT, 8)]

        def v_subs(m):
            cell = [None]
            def sub(kc):
                def go():
                    if kc == 0:
                        cell[0] = fpsum.tile([P, QW], F32, tag="f", name=f"v_{m}")
                    ps = cell[0]
                    for k in range(kc, kc + 4):
                        nc.tensor.matmul(
                            ps,
                            lhsT=xw_tiles[m][:, k, :],
                            rhs=wv_sb[:, k, :],
                            start=(k == 0),
                            stop=(k == KT - 1),
                        )
                    if kc == KT - 4:
                        nc.scalar.copy(v_sb[:, m, :], ps)
                return go
            return [sub(kc) for kc in range(0, KT, 4)]

        def p3_tile(m, n):
            def go():
                ps = fpsum.tile([P, QW], F32, tag="f")
                for h in range(NH):
                    nc.tensor.matmul(
                        ps,
                        lhsT=a_sb[:, h, m * P : (m + 1) * P],
                        rhs=wp_sb[:, h, n * QW : (n + 1) * QW],
                        start=(h == 0),
                        stop=(h == NH - 1),
                    )
                yb = ybp.tile([P, QW], BF16, tag="yb")
                # alternate the bounce copy between Scalar and Vector so the
                # final drain isn't serialized on one engine
                if (m + n) % 2 == 0:
                    nc.scalar.copy(yb, ps)
                else:
                    nc.vector.tensor_copy(yb, ps)
                nc.sync.dma_start(y[m * P : (m + 1) * P, n * QW : (n + 1) * QW], yb)
            return go

        def push_subs(key, subs):
            for i, s in enumerate(subs):
                push(key if i == len(subs) - 1 else key + ("sub", i), s)

        def push_qk(h):
            for tg in range(NQ):
                push_subs(("qk", h, tg, 0), qk_subs(h, wq_sb, qt_sb, tg))
                push_subs(("qk", h, tg, 1), qk_subs(h, wk_sb, kt_sb, tg))

        # The previous block's denominator matmul + normalize is deferred
        # until after the next block's first scores, so the PE doesn't stall
        # on the tail of the exp/accumulate pipeline at block boundaries.
        pending_fin = [None]

        def run_fin():
            if pending_fin[0] is not None:
                fn = pending_fin[0]
                pending_fin[0] = None
                fn()

        def attn_block(h, qg):
            kmax = 4 * qg + 4
            qs0 = qg * QW
            e_acc = eaccp.tile([P, QW], BF16, tag="eacc")
            ups = upsum.tile([P, QW], F32, tag="u")
            sps_tiles = [None] * kmax
            e_tiles = [None] * kmax

            def emit_score(kt):
                r = kt - 4 * qg
                c0 = 0 if r < 0 else r * P
                sps = spsum.tile([P, QW], F32, tag="s")
                nc.tensor.matmul(
                    sps[:, c0:],
                    lhsT=kt_sb[:, h, kt * P : (kt + 1) * P],
                    rhs=qt_sb[:, h, qs0 + c0 : qs0 + QW],
                    start=True,
                    stop=True,
                )
                sps_tiles[kt] = (sps, c0, r)

            def emit_post(kt):
                sps, c0, r = sps_tiles[kt]
                if r >= 0:
                    nc.vector.tensor_tensor(
                        sps[:, c0 : c0 + P], sps[:, c0 : c0 + P], mask_sb,
                        op=mybir.AluOpType.add,
                    )
                e = epool.tile([P, QW], BF16, tag="e")
                nc.scalar.activation(
                    e[:, c0:], sps[:, c0:],
                    mybir.ActivationFunctionType.Exp, scale=SCALE / (WS * WS),
                )
                if kt == 0:
                    nc.vector.tensor_copy(e_acc, e)
                else:
                    nc.vector.tensor_tensor(
                        e_acc[:, c0:], e_acc[:, c0:], e[:, c0:],
                        op=mybir.AluOpType.add,
                    )
                e_tiles[kt] = (e, c0)

            def emit_av(kt):
                e, c0 = e_tiles[kt]
                nc.tensor.matmul(
                    ups[:, c0:],
                    lhsT=v_sb[:, kt, h * HD : (h + 1) * HD],
                    rhs=e[:, c0:],
                    start=(kt == 0),
                    stop=(kt == kmax - 1),
                )

            for kt in range(min(3, kmax)):
                emit_score(kt)
                emit_post(kt)
            run_fin()
            for kt in range(kmax):
                if kt + 3 < kmax:
                    emit_score(kt + 3)
                    emit_post(kt + 3)
                emit_av(kt)
                if kt % 2 == 1:
                    drain(1)

            def fin():
                dps = dpsum.tile([1, QW], F32, tag="d")
                nc.tensor.matmul(dps, lhsT=ones_bf, rhs=e_acc, start=True, stop=True)
                rcp = rpool.tile([1, QW], F32)
                nc.vector.reciprocal_approx_fast(rcp, dps)
                rb = rbpool.tile([P, QW], F32)
                nc.gpsimd.partition_broadcast(rb, rcp)
                nc.vector.tensor_tensor(
                    a_sb[:, h, qs0 : qs0 + QW], ups, rb, op=mybir.AluOpType.mult
                )

            pending_fin[0] = fin

        # ---- emission ----
        # Wave order: for each q-group, run all 4 heads' attention blocks.
        # This spreads the V-projection and output-projection filler across
        # the whole run (v tok-tiles 4qg..4qg+3 feed wave qg; p3 for wave qg
        # becomes available during wave qg+1). Attention blocks force what
        # they need (qt/kt tokgroups <= qg, v tok-tiles <= 4qg+3) and
        # pace-drain the rest.
        for tg in range(NQ):
            for h in range(NH):
                push_subs(("qk", h, tg, 0), qk_subs(h, wq_sb, qt_sb, tg))
                push_subs(("qk", h, tg, 1), qk_subs(h, wk_sb, kt_sb, tg))
            for m in range(4 * tg, 4 * tg + 4):
                push_subs(("v", m), v_subs(m))

        def push_p3(qg):
            for m in range(4 * qg, 4 * qg + 4):
                for n in range(NQ):
                    push(("p3", m, n), p3_tile(m, n))

        for qg in range(NQ):
            for h in range(NH):
                force(("qk", h, qg, 1))
                if h == 0:
                    force(("v", 4 * qg + 3))
                attn_block(h, qg)
                if h == 0 and qg > 0:
                    # fin of wave qg-1's last block just ran inside this
                    # block; wave qg-1's a_sb rows are now final
                    push_p3(qg - 1)
        run_fin()
        push_p3(NQ - 1)
        drain(len(filler))

    nc.finalize()
    return nc


def _build_mask():
    # triangular block mask for the diagonal score tiles: scores[k_row,
    # q_col] allowed iff q >= k. Applied pre-scale — scores carry the WS^2
    # fp8 prescale, so the mask does too: exp(SCALE/WS^2 * (s + mask)).
    k = np.arange(P)[:, None]
    c = np.arange(P)[None, :]
    return np.where(c >= k, 0.0, MASK_NEG * WS * WS).astype(np.float32)


_NC_CACHE = {}


def _get_nc():
    if "nc" not in _NC_CACHE:
        _NC_CACHE["nc"] = build_bass()
    return _NC_CACHE["nc"]


def make_in_maps(x, W_qkv, W_proj):
    x = np.asarray(x, dtype=np.float32)
    W_qkv = np.asarray(W_qkv, dtype=np.float32)
    W_proj = np.asarray(W_proj, dtype=np.float32)
    Wq, Wk, Wv = W_qkv[0:D], W_qkv[D : 2 * D], W_qkv[2 * D : 3 * D]
    mask = _build_mask()
    xT_b = [np.ascontiguousarray(x[b].T) for b in range(B)]
    xbf_b = [t.astype(BF) for t in xT_b]
    x8_b = [t.astype(F8) for t in xT_b]
    in_maps = []
    for c in range(8):
        b, hg = c // HG, c % HG
        rows = slice(hg * NH * HD, (hg + 1) * NH * HD)
        in_maps.append(
            {
                "xT": xbf_b[b],
                "x8T": x8_b[b],
                "wq8": np.ascontiguousarray(WS * Wq[rows].T).astype(F8),
                "wk8": np.ascontiguousarray(WS * Wk[rows].T).astype(F8),
                "wv": np.ascontiguousarray(Wv[rows].T).astype(BF),
                "wp": np.ascontiguousarray(W_proj[:, rows].T).astype(BF),
                "mask": mask,
            }
        )
    return in_maps


def run(x, W_qkv, W_proj, trace=False):
    nc = _get_nc()
    in_maps = make_in_maps(x, W_qkv, W_proj)
    res = run_bass_kernel_spmd(nc, in_maps, core_ids=list(range(8)), trace=trace)
    out = np.zeros((B, S, D), dtype=np.float32)
    for c in range(8):
        out[c // HG] += res.results[c]["y"].astype(np.float32)
    return out, res


def kernel(x, W_qkv, W_proj):
    out, _ = run(x, W_qkv, W_proj, trace=False)
    return out
